# revision 8
# baseline (speedup 1.0000x reference)
"""Trainium2 Bass kernel for nn_CrossCorrelation.

Reference computation (per batch b of 8, c=32 channels of 128x128):
  xs = standardize(x)  (zero mean, unit "energy": / (unbiased_std * sqrt(n)))
  Xf = fft2(xs); for all ordered channel pairs (i, j>=i):
  cc = real(ifft2(Xf_i * conj(Xf_j))), rolled by (10,10), windowed to 21x21.

Device algorithm (one batch per NeuronCore, 8 cores):
  - FFTs as matmuls with DFT matrices (f32r full-rate fp32 path).
  - rfft along y (u in 0..64, Hermitian weights folded into the inverse).
  - Mean subtraction == zeroing the DC bin of the spectrum.
  - Per-channel scale folded into the P-plane copies (post x-FFT).
  - Sum(x) recovered for free from the DC column of the y-DFT.
  - Cross spectra via 3-mult Gauss complex product (DVE, bf16 2x mode).
  - Inverse transform, pair-stationary form: for each pair, the three
    product planes m_t [v=128, u=65] are the matmul STATIONARY operand
    and the small constant S-matrices [128, 42] are the moving operand:
      D^T[u, s] = sum_t sum_v m_t[v, u] * S_t[v, s]   (PSUM accumulated)
    This lands D^T directly with u on partitions - no PE transpose, no
    wide PSUM->SBUF copies. 12 pairs pack one PSUM bank (504 f32).
  - y-inverse: out = gys.T @ D^T (contract u), 21x21 window baked in.
"""

import os
import numpy as np

H = W = 128
C = 32
B = 8
NPIX = H * W
MAX_S = 10
S = 2 * MAX_S + 1  # 21
NPAIR = C * (C + 1) // 2  # 528
STD_EPS = 1e-9

UPAD = 66  # per-channel u-stride (65 used + 1 pad)
NU = 65    # rfft bins along y
BANK = 12  # pairs per PSUM bank in the inverse stage (12*42*4B = 2016B)


def _host_constants():
    import ml_dtypes

    k = np.arange(H)
    F = np.exp(-2j * np.pi * np.outer(k, k) / H)  # symmetric DFT matrix
    Fr = np.ascontiguousarray(F.real, np.float32)
    Fi = np.ascontiguousarray(F.imag, np.float32)

    # Stage A moving operand [Fr | Fi] (f32, used via f32r bitcast)
    ffs = np.concatenate([Fr, Fi], axis=1).astype(np.float32)  # (128, 256)
    # Stage B stationaries (f32): Fr, Fi, -Fi
    fmats = np.concatenate([Fr, Fi, -Fi], axis=1).astype(np.float32)  # (128, 384)

    # Inverse-side matrices. Output row s corresponds to shift (s - 10) mod 128.
    sy = (np.arange(S) - MAX_S) % H
    u = np.arange(NU)
    Gy = np.exp(2j * np.pi * np.outer(sy, u) / H)  # (21, 65)
    w_u = np.ones(NU)
    w_u[1:64] = 2.0  # Hermitian fold weights for rfft-y
    Gyw = Gy * w_u
    Gx = np.exp(2j * np.pi * np.outer(sy, np.arange(W)) / W) / NPIX  # (21, 128)

    Gxr = Gx.real.astype(np.float32)
    Gxi = Gx.imag.astype(np.float32)
    S1 = np.concatenate([Gxr, Gxi], axis=0)  # (42, 128)
    S2 = np.concatenate([-Gxi, Gxr], axis=0)
    S12 = S1 - S2
    # moving operands for the pair-stationary inverse: [S1^T | S12^T | S2^T]
    smats = np.concatenate([S1.T, S12.T, S2.T], axis=1)  # (128, 126)
    smats = smats.astype(ml_dtypes.bfloat16)

    Gywr = Gyw.real.astype(np.float32)
    Gywi = Gyw.imag.astype(np.float32)
    gys = np.concatenate([Gywr.T, (-Gywi).T], axis=1)  # (65, 42)
    gys = gys.astype(ml_dtypes.bfloat16)

    ones_col = np.ones((128, 1), np.float32)
    ones_row = np.ones((1, 128), np.float32)

    return dict(
        ffs=ffs, fmats=fmats, smats=smats, gys=gys,
        ones_col=ones_col, ones_row=ones_row,
    )


def build_nc():
    """Build the single-core Bass program (SPMD across 8 cores)."""
    import concourse.bass as bass
    import concourse.mybir as mybir
    import concourse.tile as tile
    from concourse import bacc
    from contextlib import ExitStack

    f32 = mybir.dt.float32
    f32r = mybir.dt.float32r
    bf16 = mybir.dt.bfloat16
    AF = mybir.ActivationFunctionType
    ALU = mybir.AluOpType

    nc = bacc.Bacc("TRN2", target_bir_lowering=False, debug=False)

    x_d = nc.dram_tensor("x", [C, H, W], f32r, kind="ExternalInput").ap()
    ffs_d = nc.dram_tensor("ffs", [128, 256], f32r, kind="ExternalInput").ap()
    onesc_d = nc.dram_tensor("ones_col", [128, 1], f32r, kind="ExternalInput").ap()
    onesr_d = nc.dram_tensor("ones_row", [1, 128], f32r, kind="ExternalInput").ap()
    fmats_d = nc.dram_tensor("fmats", [128, 384], f32r, kind="ExternalInput").ap()
    smats_d = nc.dram_tensor("smats", [128, 126], bf16, kind="ExternalInput").ap()
    gys_d = nc.dram_tensor("gys", [65, 42], bf16, kind="ExternalInput").ap()
    out_d = nc.dram_tensor("out", [NPAIR, S, S], f32, kind="ExternalOutput").ap()

    with tile.TileContext(nc) as tc, ExitStack() as ctx:
        cpool = ctx.enter_context(tc.tile_pool(name="consts", bufs=1))
        spool = ctx.enter_context(tc.tile_pool(name="work", bufs=1))

        # ---- constants + input loads ----
        fmats = cpool.tile([128, 384], f32r, tag="fmats")
        nc.sync.dma_start(fmats[:, :], fmats_d)
        smats = cpool.tile([128, 126], bf16, tag="smats")
        nc.sync.dma_start(smats[:, :], smats_d)
        gys = cpool.tile([65, 42], bf16, tag="gys")
        nc.sync.dma_start(gys[:, :], gys_d)
        X = spool.tile([128, C, W], f32r, tag="X")  # partition=y, free=(c, x)
        for k in range(0, C, 8):
            nc.sync.dma_start(X[:, k:k + 8, :],
                              x_d[k:k + 8].transpose([1, 0, 2]))
        ffs = cpool.tile([128, 256], f32r, tag="ffs")
        nc.sync.dma_start(ffs[:, :], ffs_d)
        ones_col = cpool.tile([128, 1], f32r, tag="ones_col")
        nc.sync.dma_start(ones_col[:, :], onesc_d)
        ones_row = cpool.tile([1, 128], f32r, tag="ones_row")
        nc.sync.dma_start(ones_row[:, :], onesr_d)

        Fr = fmats[:, 0:128]
        Fi = fmats[:, 128:256]
        Fin = fmats[:, 256:384]

        # ---- persistent SBUF work tensors ----
        # Unscaled y-DFT spectra, partition = x, free = (c, re/im, u).
        # Padded to 66 u-cols: fp32r matmul moving operands need an even
        # innermost count, so stage B consumes [.., 0:66].
        T_s = spool.tile([128, C, 2, UPAD], f32r, tag="T")
        # P-planes, one tensor so the pair products can address all four
        # with strided APs: index 0..3 = P1=(r+i)s, P2=i*s, P3=(i-r)s, P4=r*s
        P = spool.tile([128, 4, C, UPAD], bf16, tag="P")
        bc = spool.tile([128, 32], f32, tag="bc")  # per-channel scale bcast

        # zero the pad column (products read it; keep finite -> 0*0).
        # T_s needs no pad memset: the stage-A copy fills all 66 cols
        # (col 65 is the unused u=65 DFT bin, ignored downstream).
        nc.vector.memset(P[:, :, :, 65:66], 0.0)

        # =========================== phase 1 ===========================
        with tc.tile_pool(name="psA", bufs=2, space="PSUM") as psA, \
             tc.tile_pool(name="psB", bufs=2, space="PSUM") as psB, \
             tc.tile_pool(name="psS", bufs=1, space="PSUM") as psS, \
             tc.tile_pool(name="sqp", bufs=2) as sqp:

            # ---- per-channel sum(x^2) (chunked to overlap the DMA) ----
            red = spool.tile([128, 32], f32r, tag="red")
            for k in range(0, C, 8):
                s = slice(k, k + 8)
                sq = sqp.tile([128, 8, W], f32, tag="sq")
                nc.scalar.activation(sq[:, :, :], X[:, s, :], AF.Square)
                with nc.allow_low_precision(reason="f32r sum feeds f32r matmul"):
                    nc.vector.tensor_reduce(
                        red[:, k:k + 8], sq[:, :, :],
                        axis=mybir.AxisListType.X, op=ALU.add)

            # ---- stage A: y-DFT per channel: T^T = x_c^T @ [Fr|Fi] ----
            for c in range(0, C, 2):
                pa = psA.tile([128, 2, 2, 128], f32, tag="pa")
                for q in range(2):
                    pav = pa[:, q, :, :].rearrange("p a b -> p (a b)")
                    nc.tensor.matmul(pav, X[:, c + q, :], ffs[:, :],
                                     start=True, stop=True)
                nc.scalar.activation(T_s[:, c:c + 2, :, :],
                                     pa[:, :, :, 0:66], AF.Copy)

            # ---- stats: sum(x) comes free from the DC column of T ----
            stats_ps = psS.tile([1, 96], f32, tag="stats")
            # T_s[x, c, 0(re), 0] = sum_y x[y, :] ; contract x with ones
            # moving inner count must be even for fp32r: take u=0:2, the
            # sum over y lands in the even output columns
            nc.tensor.matmul(stats_ps[:, 0:64], ones_col[:, :],
                             T_s[:, :, 0, 0:2], start=True, stop=True)
            nc.tensor.matmul(stats_ps[:, 64:96], ones_col[:, :],
                             red[:, :], start=True, stop=True)

            n = float(NPIX)
            ssq = spool.tile([1, 32], f32, tag="ssq")
            nc.scalar.activation(ssq[:, :], stats_ps[:, 0:64].rearrange("p (a b) -> p a b", b=2)[:, :, 0], AF.Square)
            qn = spool.tile([1, 32], f32, tag="qn")
            nc.vector.tensor_scalar_mul(qn[:, :], stats_ps[:, 64:96], 1.0 / (n - 1.0))
            ssqs = spool.tile([1, 32], f32, tag="ssqs")
            nc.vector.tensor_scalar_mul(ssqs[:, :], ssq[:, :], -1.0 / (n * (n - 1.0)))
            var = spool.tile([1, 32], f32, tag="var")
            nc.vector.tensor_tensor(var[:, :], ssqs[:, :], qn[:, :], op=ALU.add)
            mask = spool.tile([1, 32], f32, tag="mask")
            nc.vector.tensor_scalar(mask[:, :], var[:, :], STD_EPS * STD_EPS, None,
                                    op0=ALU.is_ge)
            tn = spool.tile([1, 32], f32, tag="tn")
            nc.vector.tensor_scalar(tn[:, :], var[:, :], 1e-30, n,
                                    op0=ALU.max, op1=ALU.mult)
            rcp = spool.tile([1, 32], f32, tag="rcp")
            nc.vector.reciprocal(rcp[:, :], tn[:, :])
            rs = spool.tile([1, 32], f32, tag="rs")
            nc.scalar.sqrt(rs[:, :], rcp[:, :])  # 1/(std*sqrt(n))
            sc = spool.tile([1, 32], f32r, tag="sc")
            nc.vector.tensor_tensor(sc[:, :], rs[:, :], mask[:, :], op=ALU.mult)
            bc_ps = psS.tile([128, 32], f32, tag="bcps")
            nc.tensor.matmul(bc_ps[:, :], ones_row[:, :], sc[:, :],
                             start=True, stop=True)
            nc.scalar.copy(bc[:, :], bc_ps[:, :])

            # ---- stage B: x-DFT + scaled Gauss planes ----
            for g in range(0, C, 7):
                w = min(7, C - g)
                br = psB.tile([128, 7, UPAD], f32, tag="br")
                bi = psB.tile([128, 7, UPAD], f32, tag="bi")
                TrT = T_s[:, g:g + w, 0, :]
                TiT = T_s[:, g:g + w, 1, :]
                nc.tensor.matmul(br[:, :w, :], Fr, TrT, start=True, stop=False)
                nc.tensor.matmul(br[:, :w, :], Fin, TiT, start=False, stop=True)
                nc.tensor.matmul(bi[:, :w, :], Fi, TrT, start=True, stop=False)
                nc.tensor.matmul(bi[:, :w, :], Fr, TiT, start=False, stop=True)

                # zero each channel's DC bin [v=0,u=0] == mean subtraction
                nc.vector.memset(br[0:1, 0:w, 0:1], 0.0)
                nc.vector.memset(bi[0:1, 0:w, 0:1], 0.0)
                # P4 = r*s, P2 = i*s (per-channel scale applied here)
                for q in range(w):
                    c = g + q
                    nc.scalar.activation(P[:, 3, c, 0:65], br[:, q, 0:65], AF.Copy,
                                         scale=bc[:, c:c + 1])
                    nc.scalar.activation(P[:, 1, c, 0:65], bi[:, q, 0:65], AF.Copy,
                                         scale=bc[:, c:c + 1])
                gs = slice(g, g + w)
                nc.vector.tensor_tensor(P[:, 0, gs, 0:65], P[:, 3, gs, 0:65],
                                        P[:, 1, gs, 0:65], op=ALU.add)
                nc.vector.tensor_tensor(P[:, 2, gs, 0:65], P[:, 1, gs, 0:65],
                                        P[:, 3, gs, 0:65], op=ALU.subtract)

        # =========================== phase 2 ===========================
        BUFS = [int(v) for v in os.environ.get("K_BUFS", "4,3,3,2").split(",")]
        # copy-engine pattern per bank: A=scalar(Act), V=vector(DVE)
        DTS_PAT = os.environ.get("K_DTS", "A")
        OUT_PAT = os.environ.get("K_OUT", "A")
        # i-groups whose pair products run on gpsimd (Pool) instead of DVE
        POOL_IS = set(int(v) for v in os.environ.get("K_POOL_IS", "").split(",")
                      if v != "")

        with tc.tile_pool(name="mpool", bufs=BUFS[0]) as mpool, \
             tc.tile_pool(name="dtpool", bufs=BUFS[1]) as dtpool, \
             tc.tile_pool(name="psDT", bufs=BUFS[2], space="PSUM") as psDT, \
             tc.tile_pool(name="psO", bufs=BUFS[3], space="PSUM") as psO:

            state = dict(dt_ps=None, slot=0, bank=0)

            def flush_bank():
                nslot = state["slot"]
                if nslot == 0:
                    return
                dt_ps = state["dt_ps"]
                kbank = state["bank"]
                dts = dtpool.tile([65, BANK, 42], bf16, tag="dts")
                if DTS_PAT[kbank % len(DTS_PAT)] == "A":
                    nc.scalar.activation(dts[:, 0:nslot, :], dt_ps[:, 0:nslot, :],
                                         AF.Copy)
                else:
                    nc.vector.tensor_copy(dts[:, 0:nslot, :], dt_ps[:, 0:nslot, :])
                op_ps = psO.tile([21, BANK, 21], f32, tag="ops")
                ov = op_ps[:, 0:nslot, :]
                nc.tensor.matmul(ov, gys[:, 0:21], dts[:, 0:nslot, 0:21],
                                 start=True, stop=False)
                nc.tensor.matmul(ov, gys[:, 21:42], dts[:, 0:nslot, 21:42],
                                 start=False, stop=True)
                out_s = dtpool.tile([21, BANK, 21], f32, tag="outs")
                oc = out_s[:, 0:nslot, :]
                if OUT_PAT[kbank % len(OUT_PAT)] == "A":
                    nc.scalar.activation(oc, ov, AF.Copy)
                else:
                    nc.vector.tensor_copy(oc, ov)
                p0 = kbank * BANK
                nc.sync.dma_start(
                    out_d[p0:p0 + nslot, :, :].transpose([1, 0, 2]),
                    out_s[:, 0:nslot, :])
                state["dt_ps"] = None
                state["slot"] = 0
                state["bank"] += 1

            for i in range(C):
                npairs = C - i
                m = mpool.tile([128, 3, C, UPAD], bf16, tag="m")
                # products m[t] for t=0,1,2 = (P1*P4j, P2*P3j, P3*P2j):
                # broadcast side planes (P1,P2,P3) = P[0:3] stride +1;
                # data side planes (P4,P3,P2) = P[3:0:-1] stride -1.
                eng = nc.gpsimd if i in POOL_IS else nc.vector
                eng.tensor_tensor(
                    m[:, :, 0:npairs, :],
                    P[:, 0:3, i:i + 1, :].broadcast_to([128, 3, npairs, UPAD]),
                    P[:, 3:0:-1, i:, :],
                    op=ALU.mult)
                for j in range(i, C):
                    if state["dt_ps"] is None:
                        state["dt_ps"] = psDT.tile([65, BANK, 42], f32,
                                                   name="dt_ps", tag="dt")
                    slot = state["slot"]
                    dt_ps = state["dt_ps"]
                    for t in range(3):
                        nc.tensor.matmul(dt_ps[:, slot, :],
                                         m[:, t, j - i, 0:65],
                                         smats[:, 42 * t:42 * t + 42],
                                         start=(t == 0), stop=(t == 2))
                    state["slot"] += 1
                    if state["slot"] == BANK:
                        flush_bank()
            flush_bank()

    nc.compile()
    return nc


_CACHE = {}


def _get_nc():
    if "nc" not in _CACHE:
        _CACHE["nc"] = build_nc()
    return _CACHE["nc"]


TRACE = False  # test harness can flip this to capture an NTFF profile


def kernel(x: np.ndarray) -> np.ndarray:
    from concourse.bass_utils import run_bass_kernel_spmd

    assert x.shape == (B, C, H, W) and x.dtype == np.float32
    nc = _get_nc()
    consts = _host_constants()
    in_maps = []
    for b in range(B):
        m = {"x": np.ascontiguousarray(x[b])}
        m.update(consts)
        in_maps.append(m)
    res = run_bass_kernel_spmd(nc, in_maps, core_ids=list(range(B)), trace=TRACE)
    _CACHE["last_results"] = res
    out = np.stack([r["out"] for r in res.results]).astype(np.float32)
    return out


# revision 12
# speedup vs baseline: 1.0167x; 1.0167x over previous
"""Trainium2 Bass kernel for nn_CrossCorrelation.

Reference computation (per batch b of 8, c=32 channels of 128x128):
  xs = standardize(x)  (zero mean, unit "energy": / (unbiased_std * sqrt(n)))
  Xf = fft2(xs); for all ordered channel pairs (i, j>=i):
  cc = real(ifft2(Xf_i * conj(Xf_j))), rolled by (10,10), windowed to 21x21.

Device algorithm (one batch per NeuronCore, 8 cores):
  - FFTs as matmuls with DFT matrices (f32r full-rate fp32 path).
  - rfft along y (u in 0..64, Hermitian weights folded into the inverse).
  - Mean subtraction == zeroing the DC bin of the spectrum.
  - Per-channel scale folded into the P-plane copies (post x-FFT).
  - Sum(x) recovered for free from the DC column of the y-DFT.
  - Cross spectra via 3-mult Gauss complex product (DVE, bf16 2x mode).
  - Inverse transform, pair-stationary form: for each pair, the three
    product planes m_t [v=128, u=65] are the matmul STATIONARY operand
    and the small constant S-matrices [128, 42] are the moving operand:
      D^T[u, s] = sum_t sum_v m_t[v, u] * S_t[v, s]   (PSUM accumulated)
    This lands D^T directly with u on partitions - no PE transpose, no
    wide PSUM->SBUF copies. 12 pairs pack one PSUM bank (504 f32).
  - y-inverse: out = gys.T @ D^T (contract u), 21x21 window baked in.
"""

import os
import numpy as np

H = W = 128
C = 32
B = 8
NPIX = H * W
MAX_S = 10
S = 2 * MAX_S + 1  # 21
NPAIR = C * (C + 1) // 2  # 528
STD_EPS = 1e-9

UPAD = 66  # per-channel u-stride (65 used + 1 pad)
NU = 65    # rfft bins along y
BANK = 12  # pairs per PSUM bank in the inverse stage (12*42*4B = 2016B)


def _host_constants():
    import ml_dtypes

    k = np.arange(H)
    F = np.exp(-2j * np.pi * np.outer(k, k) / H)  # symmetric DFT matrix
    Fr = np.ascontiguousarray(F.real, np.float32)
    Fi = np.ascontiguousarray(F.imag, np.float32)

    # Stage A moving operand [Fr | Fi] (f32, used via f32r bitcast)
    ffs = np.concatenate([Fr, Fi], axis=1).astype(np.float32)  # (128, 256)
    # Stage B stationaries (f32): Fr, Fi, -Fi
    fmats = np.concatenate([Fr, Fi, -Fi], axis=1).astype(np.float32)  # (128, 384)

    # Inverse-side matrices. Output row s corresponds to shift (s - 10) mod 128.
    sy = (np.arange(S) - MAX_S) % H
    u = np.arange(NU)
    Gy = np.exp(2j * np.pi * np.outer(sy, u) / H)  # (21, 65)
    w_u = np.ones(NU)
    w_u[1:64] = 2.0  # Hermitian fold weights for rfft-y
    Gyw = Gy * w_u
    Gx = np.exp(2j * np.pi * np.outer(sy, np.arange(W)) / W) / NPIX  # (21, 128)

    Gxr = Gx.real.astype(np.float32)
    Gxi = Gx.imag.astype(np.float32)
    S1 = np.concatenate([Gxr, Gxi], axis=0)  # (42, 128)
    S2 = np.concatenate([-Gxi, Gxr], axis=0)
    S12 = S1 - S2
    # moving operands for the pair-stationary inverse: [S1^T | S12^T | S2^T]
    smats = np.concatenate([S1.T, S12.T, S2.T], axis=1)  # (128, 126)
    smats = smats.astype(ml_dtypes.bfloat16)

    Gywr = Gyw.real.astype(np.float32)
    Gywi = Gyw.imag.astype(np.float32)
    gys = np.concatenate([Gywr.T, (-Gywi).T], axis=1)  # (65, 42)
    gys = gys.astype(ml_dtypes.bfloat16)

    ones_col = np.ones((128, 1), np.float32)
    ones_row = np.ones((1, 128), np.float32)

    return dict(
        ffs=ffs, fmats=fmats, smats=smats, gys=gys,
        ones_col=ones_col, ones_row=ones_row,
    )


def build_nc():
    """Build the single-core Bass program (SPMD across 8 cores)."""
    import concourse.bass as bass
    import concourse.mybir as mybir
    import concourse.tile as tile
    from concourse import bacc
    from contextlib import ExitStack

    f32 = mybir.dt.float32
    f32r = mybir.dt.float32r
    bf16 = mybir.dt.bfloat16
    AF = mybir.ActivationFunctionType
    ALU = mybir.AluOpType

    nc = bacc.Bacc("TRN2", target_bir_lowering=False, debug=False)

    x_d = nc.dram_tensor("x", [C, H, W], f32r, kind="ExternalInput").ap()
    ffs_d = nc.dram_tensor("ffs", [128, 256], f32r, kind="ExternalInput").ap()
    onesc_d = nc.dram_tensor("ones_col", [128, 1], f32r, kind="ExternalInput").ap()
    onesr_d = nc.dram_tensor("ones_row", [1, 128], f32r, kind="ExternalInput").ap()
    fmats_d = nc.dram_tensor("fmats", [128, 384], f32r, kind="ExternalInput").ap()
    smats_d = nc.dram_tensor("smats", [128, 126], bf16, kind="ExternalInput").ap()
    gys_d = nc.dram_tensor("gys", [65, 42], bf16, kind="ExternalInput").ap()
    out_d = nc.dram_tensor("out", [NPAIR, S, S], f32, kind="ExternalOutput").ap()

    with tile.TileContext(nc) as tc, ExitStack() as ctx:
        cpool = ctx.enter_context(tc.tile_pool(name="consts", bufs=1))
        spool = ctx.enter_context(tc.tile_pool(name="work", bufs=1))

        # ---- constants + input loads ----
        fmats = cpool.tile([128, 384], f32r, tag="fmats")
        nc.sync.dma_start(fmats[:, :], fmats_d)
        smats = cpool.tile([128, 126], bf16, tag="smats")
        nc.sync.dma_start(smats[:, :], smats_d)
        gys = cpool.tile([65, 42], bf16, tag="gys")
        nc.sync.dma_start(gys[:, :], gys_d)
        X = spool.tile([128, C, W], f32r, tag="X")  # partition=y, free=(c, x)
        for k in range(0, C, 4):
            nc.sync.dma_start(X[:, k:k + 4, :],
                              x_d[k:k + 4].transpose([1, 0, 2]))
        ffs = cpool.tile([128, 256], f32r, tag="ffs")
        nc.sync.dma_start(ffs[:, :], ffs_d)
        ones_col = cpool.tile([128, 1], f32r, tag="ones_col")
        nc.sync.dma_start(ones_col[:, :], onesc_d)
        ones_row = cpool.tile([1, 128], f32r, tag="ones_row")
        nc.sync.dma_start(ones_row[:, :], onesr_d)

        Fr = fmats[:, 0:128]
        Fi = fmats[:, 128:256]
        Fin = fmats[:, 256:384]

        # ---- persistent SBUF work tensors ----
        # Unscaled y-DFT spectra, partition = x, free = (c, re/im, u).
        # Padded to 66 u-cols: fp32r matmul moving operands need an even
        # innermost count, so stage B consumes [.., 0:66].
        T_s = spool.tile([128, C, 2, UPAD], f32r, tag="T")
        # P-planes, one tensor so the pair products can address all four
        # with strided APs: index 0..3 = P1=(r+i)s, P2=i*s, P3=(i-r)s, P4=r*s
        P = spool.tile([128, 4, C, UPAD], bf16, tag="P")
        bc = spool.tile([128, 32], f32, tag="bc")  # per-channel scale bcast

        # No pad memsets needed: products read u-cols 0:65 only, and the
        # stage-A copy fills all 66 T_s cols (col 65 = unused u=65 DFT bin).

        # =========================== phase 1 ===========================
        with tc.tile_pool(name="psA", bufs=2, space="PSUM") as psA, \
             tc.tile_pool(name="psB", bufs=2, space="PSUM") as psB, \
             tc.tile_pool(name="psS", bufs=1, space="PSUM") as psS, \
             tc.tile_pool(name="sqp", bufs=2) as sqp, \
             tc.tile_pool(name="prawp", bufs=2) as prawp:

            # ---- per-channel sum(x^2) (chunked to overlap the DMA) ----
            red = spool.tile([128, 32], f32r, tag="red")
            for k in range(0, C, 8):
                s = slice(k, k + 8)
                sq = sqp.tile([128, 8, W], f32, tag="sq")
                nc.scalar.activation(sq[:, :, :], X[:, s, :], AF.Square)
                with nc.allow_low_precision(reason="f32r sum feeds f32r matmul"):
                    nc.vector.tensor_reduce(
                        red[:, k:k + 8], sq[:, :, :],
                        axis=mybir.AxisListType.X, op=ALU.add)

            # ---- stage A: y-DFT per channel: T^T = x_c^T @ [Fr|Fi] ----
            for c in range(0, C, 2):
                pa = psA.tile([128, 2, 2, 128], f32, tag="pa")
                for q in range(2):
                    pav = pa[:, q, :, :].rearrange("p a b -> p (a b)")
                    nc.tensor.matmul(pav, X[:, c + q, :], ffs[:, :],
                                     start=True, stop=True)
                nc.scalar.activation(T_s[:, c:c + 2, :, :],
                                     pa[:, :, :, 0:66], AF.Copy)

            # ---- stats: sum(x) comes free from the DC column of T ----
            stats_ps = psS.tile([1, 96], f32, tag="stats")
            # T_s[x, c, 0(re), 0] = sum_y x[y, :] ; contract x with ones
            # moving inner count must be even for fp32r: take u=0:2, the
            # sum over y lands in the even output columns
            nc.tensor.matmul(stats_ps[:, 0:64], ones_col[:, :],
                             T_s[:, :, 0, 0:2], start=True, stop=True)
            nc.tensor.matmul(stats_ps[:, 64:96], ones_col[:, :],
                             red[:, :], start=True, stop=True)

            n = float(NPIX)
            ssq = spool.tile([1, 32], f32, tag="ssq")
            nc.scalar.activation(ssq[:, :], stats_ps[:, 0:64].rearrange("p (a b) -> p a b", b=2)[:, :, 0], AF.Square)
            qn = spool.tile([1, 32], f32, tag="qn")
            nc.vector.tensor_scalar_mul(qn[:, :], stats_ps[:, 64:96], 1.0 / (n - 1.0))
            ssqs = spool.tile([1, 32], f32, tag="ssqs")
            nc.vector.tensor_scalar_mul(ssqs[:, :], ssq[:, :], -1.0 / (n * (n - 1.0)))
            var = spool.tile([1, 32], f32, tag="var")
            nc.vector.tensor_tensor(var[:, :], ssqs[:, :], qn[:, :], op=ALU.add)
            mask = spool.tile([1, 32], f32, tag="mask")
            nc.vector.tensor_scalar(mask[:, :], var[:, :], STD_EPS * STD_EPS, None,
                                    op0=ALU.is_ge)
            tn = spool.tile([1, 32], f32, tag="tn")
            nc.vector.tensor_scalar(tn[:, :], var[:, :], 1e-30, n,
                                    op0=ALU.max, op1=ALU.mult)
            rcp = spool.tile([1, 32], f32, tag="rcp")
            nc.vector.reciprocal(rcp[:, :], tn[:, :])
            rs = spool.tile([1, 32], f32, tag="rs")
            nc.scalar.sqrt(rs[:, :], rcp[:, :])  # 1/(std*sqrt(n))
            sc = spool.tile([1, 32], f32r, tag="sc")
            nc.vector.tensor_tensor(sc[:, :], rs[:, :], mask[:, :], op=ALU.mult)
            bc_ps = psS.tile([128, 32], f32, tag="bcps")
            nc.tensor.matmul(bc_ps[:, :], ones_row[:, :], sc[:, :],
                             start=True, stop=True)
            nc.scalar.copy(bc[:, :], bc_ps[:, :])

            # ---- stage B: x-DFT + scaled Gauss planes ----
            for g in range(0, C, 7):
                w = min(7, C - g)
                br = psB.tile([128, 7, UPAD], f32, tag="br")
                bi = psB.tile([128, 7, UPAD], f32, tag="bi")
                TrT = T_s[:, g:g + w, 0, :]
                TiT = T_s[:, g:g + w, 1, :]
                nc.tensor.matmul(br[:, :w, :], Fr, TrT, start=True, stop=False)
                nc.tensor.matmul(br[:, :w, :], Fin, TiT, start=False, stop=True)
                nc.tensor.matmul(bi[:, :w, :], Fi, TrT, start=True, stop=False)
                nc.tensor.matmul(bi[:, :w, :], Fr, TiT, start=False, stop=True)

                # batched raw copies PSUM->SBUF (gpsimd cannot touch PSUM)
                praw = prawp.tile([128, 2, 7, 65], bf16, tag="praw")
                nc.scalar.activation(praw[:, 0, 0:w, :], br[:, 0:w, 0:65],
                                     AF.Copy)
                nc.scalar.activation(praw[:, 1, 0:w, :], bi[:, 0:w, 0:65],
                                     AF.Copy)
                # zero each channel's DC bin [v=0,u=0] == mean subtraction
                nc.gpsimd.memset(praw[0:1, :, 0:w, 0:1], 0.0)
                # P4 = r*s, P2 = i*s: per-channel scale on the idle gpsimd
                for q in range(w):
                    c = g + q
                    nc.gpsimd.tensor_scalar_mul(P[:, 3, c, 0:65],
                                                praw[:, 0, q, :], bc[:, c:c + 1])
                    nc.gpsimd.tensor_scalar_mul(P[:, 1, c, 0:65],
                                                praw[:, 1, q, :], bc[:, c:c + 1])
                gs = slice(g, g + w)
                nc.vector.tensor_tensor(P[:, 0, gs, 0:65], P[:, 3, gs, 0:65],
                                        P[:, 1, gs, 0:65], op=ALU.add)
                nc.vector.tensor_tensor(P[:, 2, gs, 0:65], P[:, 1, gs, 0:65],
                                        P[:, 3, gs, 0:65], op=ALU.subtract)

        # =========================== phase 2 ===========================
        BUFS = [int(v) for v in os.environ.get("K_BUFS", "8,3,5,2").split(",")]
        # copy-engine pattern: A=scalar(Act), V=vector(DVE)
        DTS_PAT = os.environ.get("K_DTS", "A")
        OUT_PAT = os.environ.get("K_OUT", "A")
        # every Nth product block runs on gpsimd (0 = never)
        POOL_EVERY = int(os.environ.get("K_POOL_EVERY", "0"))

        GW = 7     # channel-group width (matches stage B groups)
        SB = 2     # psDT banks per output super-bank (24 pairs)

        with tc.tile_pool(name="mpool", bufs=BUFS[0]) as mpool, \
             tc.tile_pool(name="dtpool", bufs=BUFS[1]) as dtpool, \
             tc.tile_pool(name="psDT", bufs=BUFS[2], space="PSUM") as psDT, \
             tc.tile_pool(name="psO", bufs=BUFS[3], space="PSUM") as psO:

            state = dict(banks=[], slot=0, super=0, blk=0)

            def flush_super():
                nslot = state["slot"]
                if nslot == 0:
                    return
                ksuper = state["super"]
                nb = len(state["banks"])
                dts = dtpool.tile([65, SB * BANK, 42], bf16, tag="dts")
                for h, bank in enumerate(state["banks"]):
                    wb = min(BANK, nslot - h * BANK)
                    ce = DTS_PAT[(ksuper * SB + h) % len(DTS_PAT)]
                    dst = dts[:, h * BANK:h * BANK + wb, :]
                    if ce == "A":
                        nc.scalar.activation(dst, bank[:, 0:wb, :], AF.Copy)
                    else:
                        nc.vector.tensor_copy(dst, bank[:, 0:wb, :])
                op_ps = psO.tile([21, SB * BANK, 21], f32, tag="ops")
                ov = op_ps[:, 0:nslot, :]
                nc.tensor.matmul(ov, gys[:, 0:21], dts[:, 0:nslot, 0:21],
                                 start=True, stop=False)
                nc.tensor.matmul(ov, gys[:, 21:42], dts[:, 0:nslot, 21:42],
                                 start=False, stop=True)
                out_s = dtpool.tile([21, SB * BANK, 21], f32, tag="outs")
                oc = out_s[:, 0:nslot, :]
                if OUT_PAT[ksuper % len(OUT_PAT)] == "A":
                    nc.scalar.activation(oc, ov, AF.Copy)
                else:
                    nc.vector.tensor_copy(oc, ov)
                p0 = ksuper * SB * BANK
                nc.sync.dma_start(
                    out_d[p0:p0 + nslot, :, :].transpose([1, 0, 2]),
                    out_s[:, 0:nslot, :])
                state["banks"] = []
                state["slot"] = 0
                state["super"] += 1

            for i in range(C):
                # products per (i, j-block): a block's products only need
                # that block's P-planes, so phase 2 starts as soon as the
                # first stage-B group is done (not after all of stage B).
                for jb in range(i // GW * GW, C, GW):
                    j0 = max(i, jb)
                    j1 = min(C, jb + GW)
                    nb = j1 - j0
                    m = mpool.tile([128, 3, GW, 65], bf16, tag="m")
                    # products m[t] for t=0,1,2 = (P1*P4j, P2*P3j, P3*P2j):
                    # broadcast side planes (P1,P2,P3) = P[0:3] stride +1;
                    # data side planes (P4,P3,P2) = P[3:0:-1] stride -1.
                    state["blk"] += 1
                    pool_blk = POOL_EVERY and state["blk"] % POOL_EVERY == 0
                    eng = nc.gpsimd if pool_blk else nc.vector
                    eng.tensor_tensor(
                        m[:, :, 0:nb, :],
                        P[:, 0:3, i:i + 1, 0:65].broadcast_to([128, 3, nb, 65]),
                        P[:, 3:0:-1, j0:j1, 0:65],
                        op=ALU.mult)
                    for j in range(j0, j1):
                        slot = state["slot"]
                        if slot % BANK == 0:
                            state["banks"].append(
                                psDT.tile([65, BANK, 42], f32,
                                          name="dt_ps", tag="dt"))
                        dt_ps = state["banks"][-1]
                        for t in range(3):
                            nc.tensor.matmul(dt_ps[:, slot % BANK, :],
                                             m[:, t, j - j0, 0:65],
                                             smats[:, 42 * t:42 * t + 42],
                                             start=(t == 0), stop=(t == 2))
                        state["slot"] += 1
                        if state["slot"] == SB * BANK:
                            flush_super()
            flush_super()

    nc.compile()
    return nc


_CACHE = {}


def _get_nc():
    if "nc" not in _CACHE:
        _CACHE["nc"] = build_nc()
    return _CACHE["nc"]


TRACE = False  # test harness can flip this to capture an NTFF profile


def kernel(x: np.ndarray) -> np.ndarray:
    from concourse.bass_utils import run_bass_kernel_spmd

    assert x.shape == (B, C, H, W) and x.dtype == np.float32
    nc = _get_nc()
    consts = _host_constants()
    in_maps = []
    for b in range(B):
        m = {"x": np.ascontiguousarray(x[b])}
        m.update(consts)
        in_maps.append(m)
    res = run_bass_kernel_spmd(nc, in_maps, core_ids=list(range(B)), trace=TRACE)
    _CACHE["last_results"] = res
    out = np.stack([r["out"] for r in res.results]).astype(np.float32)
    return out


# revision 13
# speedup vs baseline: 1.1410x; 1.1222x over previous
"""Trainium2 Bass kernel for nn_CrossCorrelation.

Reference computation (per batch b of 8, c=32 channels of 128x128):
  xs = standardize(x)  (zero mean, unit "energy": / (unbiased_std * sqrt(n)))
  Xf = fft2(xs); for all ordered channel pairs (i, j>=i):
  cc = real(ifft2(Xf_i * conj(Xf_j))), rolled by (10,10), windowed to 21x21.

Device algorithm (one batch per NeuronCore, 8 cores):
  - FFTs as matmuls with DFT matrices (f32r full-rate fp32 path).
  - rfft along y (u in 0..64, Hermitian weights folded into the inverse).
  - Mean subtraction == zeroing the DC bin of the spectrum.
  - Per-channel scale folded into the P-plane copies (post x-FFT).
  - Sum(x) recovered for free from the DC column of the y-DFT.
  - Cross spectra via 3-mult Gauss complex product (DVE, bf16 2x mode).
  - Inverse transform, pair-stationary form: for each pair, the three
    product planes m_t [v=128, u=65] are the matmul STATIONARY operand
    and the small constant S-matrices [128, 42] are the moving operand:
      D^T[u, s] = sum_t sum_v m_t[v, u] * S_t[v, s]   (PSUM accumulated)
    This lands D^T directly with u on partitions - no PE transpose, no
    wide PSUM->SBUF copies. 12 pairs pack one PSUM bank (504 f32).
  - y-inverse: out = gys.T @ D^T (contract u), 21x21 window baked in.
"""

import os
import numpy as np

H = W = 128
C = 32
B = 8
NPIX = H * W
MAX_S = 10
S = 2 * MAX_S + 1  # 21
NPAIR = C * (C + 1) // 2  # 528
STD_EPS = 1e-9

UPAD = 66  # per-channel u-stride (65 used + 1 pad)
NU = 65    # rfft bins along y
BANK = 12  # pairs per PSUM bank in the inverse stage (12*42*4B = 2016B)


def _host_constants():
    import ml_dtypes

    k = np.arange(H)
    F = np.exp(-2j * np.pi * np.outer(k, k) / H)  # symmetric DFT matrix
    Fr = np.ascontiguousarray(F.real, np.float32)
    Fi = np.ascontiguousarray(F.imag, np.float32)

    # Stage A moving operand [Fr | Fi] (f32, used via f32r bitcast)
    ffs = np.concatenate([Fr, Fi], axis=1).astype(np.float32)  # (128, 256)
    # Stage B stationaries (f32): Fr, Fi, -Fi
    fmats = np.concatenate([Fr, Fi, -Fi], axis=1).astype(np.float32)  # (128, 384)

    # Inverse-side matrices. Output row s corresponds to shift (s - 10) mod 128.
    sy = (np.arange(S) - MAX_S) % H
    u = np.arange(NU)
    Gy = np.exp(2j * np.pi * np.outer(sy, u) / H)  # (21, 65)
    w_u = np.ones(NU)
    w_u[1:64] = 2.0  # Hermitian fold weights for rfft-y
    Gyw = Gy * w_u
    Gx = np.exp(2j * np.pi * np.outer(sy, np.arange(W)) / W) / NPIX  # (21, 128)

    Gxr = Gx.real.astype(np.float32)
    Gxi = Gx.imag.astype(np.float32)
    S1 = np.concatenate([Gxr, Gxi], axis=0)  # (42, 128)
    S2 = np.concatenate([-Gxi, Gxr], axis=0)
    S12 = S1 - S2
    # moving operands for the pair-stationary inverse: [S1^T | S12^T | S2^T]
    smats = np.concatenate([S1.T, S12.T, S2.T], axis=1)  # (128, 126)
    smats = smats.astype(ml_dtypes.bfloat16)

    Gywr = Gyw.real.astype(np.float32)
    Gywi = Gyw.imag.astype(np.float32)
    gys = np.concatenate([Gywr.T, (-Gywi).T], axis=1)  # (65, 42)
    gys = gys.astype(ml_dtypes.bfloat16)

    ones_col = np.ones((128, 1), np.float32)
    ones_row = np.ones((1, 128), np.float32)

    return dict(
        ffs=ffs, fmats=fmats, smats=smats, gys=gys,
        ones_col=ones_col, ones_row=ones_row,
    )


def build_nc():
    """Build the single-core Bass program (SPMD across 8 cores)."""
    import concourse.bass as bass
    import concourse.mybir as mybir
    import concourse.tile as tile
    from concourse import bacc
    from contextlib import ExitStack

    f32 = mybir.dt.float32
    f32r = mybir.dt.float32r
    bf16 = mybir.dt.bfloat16
    AF = mybir.ActivationFunctionType
    ALU = mybir.AluOpType

    nc = bacc.Bacc("TRN2", target_bir_lowering=False, debug=False)

    x_d = nc.dram_tensor("x", [C, H, W], f32r, kind="ExternalInput").ap()
    ffs_d = nc.dram_tensor("ffs", [128, 256], f32r, kind="ExternalInput").ap()
    onesc_d = nc.dram_tensor("ones_col", [128, 1], f32r, kind="ExternalInput").ap()
    onesr_d = nc.dram_tensor("ones_row", [1, 128], f32r, kind="ExternalInput").ap()
    fmats_d = nc.dram_tensor("fmats", [128, 384], f32r, kind="ExternalInput").ap()
    smats_d = nc.dram_tensor("smats", [128, 126], bf16, kind="ExternalInput").ap()
    gys_d = nc.dram_tensor("gys", [65, 42], bf16, kind="ExternalInput").ap()
    out_d = nc.dram_tensor("out", [NPAIR, S, S], f32, kind="ExternalOutput").ap()

    with tile.TileContext(nc) as tc, ExitStack() as ctx:
        cpool = ctx.enter_context(tc.tile_pool(name="consts", bufs=1))
        spool = ctx.enter_context(tc.tile_pool(name="work", bufs=1))

        # ---- constants + input loads (consts first: stage A needs ffs
        # before the first X chunk, and the SP queue is in-order) ----
        ffs = cpool.tile([128, 256], f32r, tag="ffs")
        nc.sync.dma_start(ffs[:, :], ffs_d)
        fmats = cpool.tile([128, 384], f32r, tag="fmats")
        nc.sync.dma_start(fmats[:, :], fmats_d)
        smats = cpool.tile([128, 126], bf16, tag="smats")
        nc.sync.dma_start(smats[:, :], smats_d)
        gys = cpool.tile([65, 42], bf16, tag="gys")
        nc.sync.dma_start(gys[:, :], gys_d)
        ones_col = cpool.tile([128, 1], f32r, tag="ones_col")
        nc.sync.dma_start(ones_col[:, :], onesc_d)
        ones_row = cpool.tile([1, 128], f32r, tag="ones_row")
        nc.sync.dma_start(ones_row[:, :], onesr_d)
        X = spool.tile([128, C, W], f32r, tag="X")  # partition=y, free=(c, x)
        for k in range(0, C, 4):
            nc.sync.dma_start(X[:, k:k + 4, :],
                              x_d[k:k + 4].transpose([1, 0, 2]))

        Fr = fmats[:, 0:128]
        Fi = fmats[:, 128:256]
        Fin = fmats[:, 256:384]

        # ---- persistent SBUF work tensors ----
        # Unscaled y-DFT spectra, partition = x, free = (c, re/im, u).
        # Padded to 66 u-cols: fp32r matmul moving operands need an even
        # innermost count, so stage B consumes [.., 0:66].
        T_s = spool.tile([128, C, 2, UPAD], f32r, tag="T")
        # P-planes, one tensor so the pair products can address all four
        # with strided APs: index 0..3 = P1=(r+i)s, P2=i*s, P3=(i-r)s, P4=r*s
        P = spool.tile([128, 4, C, UPAD], bf16, tag="P")
        bc = spool.tile([128, 32], f32, tag="bc")  # per-channel scale bcast

        # No pad memsets needed: products read u-cols 0:65 only, and the
        # stage-A copy fills all 66 T_s cols (col 65 = unused u=65 DFT bin).

        # =========================== phase 1 ===========================
        with tc.tile_pool(name="psA", bufs=2, space="PSUM") as psA, \
             tc.tile_pool(name="psB", bufs=2, space="PSUM") as psB, \
             tc.tile_pool(name="psS", bufs=1, space="PSUM") as psS, \
             tc.tile_pool(name="sqp", bufs=2) as sqp, \
             tc.tile_pool(name="prawp", bufs=2) as prawp:

            # ---- per-channel sum(x^2) (chunked to overlap the DMA) ----
            red = spool.tile([128, 32], f32r, tag="red")
            for k in range(0, C, 8):
                s = slice(k, k + 8)
                sq = sqp.tile([128, 8, W], f32, tag="sq")
                nc.gpsimd.tensor_tensor(sq[:, :, :], X[:, s, :], X[:, s, :],
                                        op=ALU.mult)
                with nc.allow_low_precision(reason="f32r sum feeds f32r matmul"):
                    nc.vector.tensor_reduce(
                        red[:, k:k + 8], sq[:, :, :],
                        axis=mybir.AxisListType.X, op=ALU.add)

            # ---- stage A: y-DFT per channel: T^T = x_c^T @ [Fr|Fi] ----
            for c in range(0, C, 2):
                pa = psA.tile([128, 2, 2, 128], f32, tag="pa")
                for q in range(2):
                    pav = pa[:, q, :, :].rearrange("p a b -> p (a b)")
                    nc.tensor.matmul(pav, X[:, c + q, :], ffs[:, :],
                                     start=True, stop=True)
                nc.scalar.activation(T_s[:, c:c + 2, :, :],
                                     pa[:, :, :, 0:66], AF.Copy)

            # ---- stats: sum(x) comes free from the DC column of T ----
            stats_ps = psS.tile([1, 96], f32, tag="stats")
            # T_s[x, c, 0(re), 0] = sum_y x[y, :] ; contract x with ones
            # moving inner count must be even for fp32r: take u=0:2, the
            # sum over y lands in the even output columns
            nc.tensor.matmul(stats_ps[:, 0:64], ones_col[:, :],
                             T_s[:, :, 0, 0:2], start=True, stop=True)
            nc.tensor.matmul(stats_ps[:, 64:96], ones_col[:, :],
                             red[:, :], start=True, stop=True)

            n = float(NPIX)
            ssq = spool.tile([1, 32], f32, tag="ssq")
            nc.scalar.activation(ssq[:, :], stats_ps[:, 0:64].rearrange("p (a b) -> p a b", b=2)[:, :, 0], AF.Square)
            qn = spool.tile([1, 32], f32, tag="qn")
            nc.vector.tensor_scalar_mul(qn[:, :], stats_ps[:, 64:96], 1.0 / (n - 1.0))
            ssqs = spool.tile([1, 32], f32, tag="ssqs")
            nc.vector.tensor_scalar_mul(ssqs[:, :], ssq[:, :], -1.0 / (n * (n - 1.0)))
            var = spool.tile([1, 32], f32, tag="var")
            nc.vector.tensor_tensor(var[:, :], ssqs[:, :], qn[:, :], op=ALU.add)
            mask = spool.tile([1, 32], f32, tag="mask")
            nc.vector.tensor_scalar(mask[:, :], var[:, :], STD_EPS * STD_EPS, None,
                                    op0=ALU.is_ge)
            tn = spool.tile([1, 32], f32, tag="tn")
            nc.vector.tensor_scalar(tn[:, :], var[:, :], 1e-30, n,
                                    op0=ALU.max, op1=ALU.mult)
            rcp = spool.tile([1, 32], f32, tag="rcp")
            nc.vector.reciprocal(rcp[:, :], tn[:, :])
            rs = spool.tile([1, 32], f32, tag="rs")
            nc.scalar.sqrt(rs[:, :], rcp[:, :])  # 1/(std*sqrt(n))
            sc = spool.tile([1, 32], f32r, tag="sc")
            nc.vector.tensor_tensor(sc[:, :], rs[:, :], mask[:, :], op=ALU.mult)
            bc_ps = psS.tile([128, 32], f32, tag="bcps")
            nc.tensor.matmul(bc_ps[:, :], ones_row[:, :], sc[:, :],
                             start=True, stop=True)
            nc.scalar.copy(bc[:, :], bc_ps[:, :])

            # ---- stage B: x-DFT + scaled Gauss planes ----
            for g in range(0, C, 7):
                w = min(7, C - g)
                br = psB.tile([128, 7, UPAD], f32, tag="br")
                bi = psB.tile([128, 7, UPAD], f32, tag="bi")
                TrT = T_s[:, g:g + w, 0, :]
                TiT = T_s[:, g:g + w, 1, :]
                nc.tensor.matmul(br[:, :w, :], Fr, TrT, start=True, stop=False)
                nc.tensor.matmul(br[:, :w, :], Fin, TiT, start=False, stop=True)
                nc.tensor.matmul(bi[:, :w, :], Fi, TrT, start=True, stop=False)
                nc.tensor.matmul(bi[:, :w, :], Fr, TiT, start=False, stop=True)

                # batched raw copies PSUM->SBUF (gpsimd cannot touch PSUM)
                praw = prawp.tile([128, 2, 7, 65], bf16, tag="praw")
                nc.scalar.activation(praw[:, 0, 0:w, :], br[:, 0:w, 0:65],
                                     AF.Copy)
                nc.scalar.activation(praw[:, 1, 0:w, :], bi[:, 0:w, 0:65],
                                     AF.Copy)
                # zero each channel's DC bin [v=0,u=0] == mean subtraction
                nc.gpsimd.memset(praw[0:1, :, 0:w, 0:1], 0.0)
                # P4 = r*s, P2 = i*s: per-channel scale; DVE tensor_scalar
                # runs in 4x mode on bf16 SBUF operands (~77ns/op)
                for q in range(w):
                    c = g + q
                    nc.vector.tensor_scalar_mul(P[:, 3, c, 0:65],
                                                praw[:, 0, q, :], bc[:, c:c + 1])
                    nc.vector.tensor_scalar_mul(P[:, 1, c, 0:65],
                                                praw[:, 1, q, :], bc[:, c:c + 1])
                gs = slice(g, g + w)
                nc.vector.tensor_tensor(P[:, 0, gs, 0:65], P[:, 3, gs, 0:65],
                                        P[:, 1, gs, 0:65], op=ALU.add)
                nc.vector.tensor_tensor(P[:, 2, gs, 0:65], P[:, 1, gs, 0:65],
                                        P[:, 3, gs, 0:65], op=ALU.subtract)

        # =========================== phase 2 ===========================
        BUFS = [int(v) for v in os.environ.get("K_BUFS", "8,3,5,2").split(",")]
        # copy-engine pattern: A=scalar(Act), V=vector(DVE)
        DTS_PAT = os.environ.get("K_DTS", "A")
        OUT_PAT = os.environ.get("K_OUT", "A")
        # every Nth product block runs on gpsimd (0 = never)
        POOL_EVERY = int(os.environ.get("K_POOL_EVERY", "10"))

        GW = 7     # channel-group width (matches stage B groups)
        SB = 2     # psDT banks per output super-bank (24 pairs)

        with tc.tile_pool(name="mpool", bufs=BUFS[0]) as mpool, \
             tc.tile_pool(name="dtpool", bufs=BUFS[1]) as dtpool, \
             tc.tile_pool(name="psDT", bufs=BUFS[2], space="PSUM") as psDT, \
             tc.tile_pool(name="psO", bufs=BUFS[3], space="PSUM") as psO:

            state = dict(banks=[], slot=0, super=0, blk=0)

            def flush_super():
                nslot = state["slot"]
                if nslot == 0:
                    return
                ksuper = state["super"]
                nb = len(state["banks"])
                dts = dtpool.tile([65, SB * BANK, 42], bf16, tag="dts")
                for h, bank in enumerate(state["banks"]):
                    wb = min(BANK, nslot - h * BANK)
                    ce = DTS_PAT[(ksuper * SB + h) % len(DTS_PAT)]
                    dst = dts[:, h * BANK:h * BANK + wb, :]
                    if ce == "A":
                        nc.scalar.activation(dst, bank[:, 0:wb, :], AF.Copy)
                    else:
                        nc.vector.tensor_copy(dst, bank[:, 0:wb, :])
                op_ps = psO.tile([21, SB * BANK, 21], f32, tag="ops")
                ov = op_ps[:, 0:nslot, :]
                nc.tensor.matmul(ov, gys[:, 0:21], dts[:, 0:nslot, 0:21],
                                 start=True, stop=False)
                nc.tensor.matmul(ov, gys[:, 21:42], dts[:, 0:nslot, 21:42],
                                 start=False, stop=True)
                out_s = dtpool.tile([21, SB * BANK, 21], f32, tag="outs")
                oc = out_s[:, 0:nslot, :]
                if OUT_PAT[ksuper % len(OUT_PAT)] == "A":
                    nc.scalar.activation(oc, ov, AF.Copy)
                else:
                    nc.vector.tensor_copy(oc, ov)
                p0 = ksuper * SB * BANK
                nc.sync.dma_start(
                    out_d[p0:p0 + nslot, :, :].transpose([1, 0, 2]),
                    out_s[:, 0:nslot, :])
                state["banks"] = []
                state["slot"] = 0
                state["super"] += 1

            for i in range(C):
                # products per (i, j-block): a block's products only need
                # that block's P-planes, so phase 2 starts as soon as the
                # first stage-B group is done (not after all of stage B).
                for jb in range(i // GW * GW, C, GW):
                    j0 = max(i, jb)
                    j1 = min(C, jb + GW)
                    nb = j1 - j0
                    m = mpool.tile([128, 3, GW, 65], bf16, tag="m")
                    # products m[t] for t=0,1,2 = (P1*P4j, P2*P3j, P3*P2j):
                    # broadcast side planes (P1,P2,P3) = P[0:3] stride +1;
                    # data side planes (P4,P3,P2) = P[3:0:-1] stride -1.
                    state["blk"] += 1
                    pool_blk = POOL_EVERY and state["blk"] % POOL_EVERY == 0
                    eng = nc.gpsimd if pool_blk else nc.vector
                    eng.tensor_tensor(
                        m[:, :, 0:nb, :],
                        P[:, 0:3, i:i + 1, 0:65].broadcast_to([128, 3, nb, 65]),
                        P[:, 3:0:-1, j0:j1, 0:65],
                        op=ALU.mult)
                    for j in range(j0, j1):
                        slot = state["slot"]
                        if slot % BANK == 0:
                            state["banks"].append(
                                psDT.tile([65, BANK, 42], f32,
                                          name="dt_ps", tag="dt"))
                        dt_ps = state["banks"][-1]
                        for t in range(3):
                            nc.tensor.matmul(dt_ps[:, slot % BANK, :],
                                             m[:, t, j - j0, 0:65],
                                             smats[:, 42 * t:42 * t + 42],
                                             start=(t == 0), stop=(t == 2))
                        state["slot"] += 1
                        if state["slot"] == SB * BANK:
                            flush_super()
            flush_super()

    nc.compile()
    return nc


_CACHE = {}


def _get_nc():
    if "nc" not in _CACHE:
        _CACHE["nc"] = build_nc()
    return _CACHE["nc"]


TRACE = False  # test harness can flip this to capture an NTFF profile


def kernel(x: np.ndarray) -> np.ndarray:
    from concourse.bass_utils import run_bass_kernel_spmd

    assert x.shape == (B, C, H, W) and x.dtype == np.float32
    nc = _get_nc()
    consts = _host_constants()
    in_maps = []
    for b in range(B):
        m = {"x": np.ascontiguousarray(x[b])}
        m.update(consts)
        in_maps.append(m)
    res = run_bass_kernel_spmd(nc, in_maps, core_ids=list(range(B)), trace=TRACE)
    _CACHE["last_results"] = res
    out = np.stack([r["out"] for r in res.results]).astype(np.float32)
    return out
